# revision 1
# baseline (speedup 1.0000x reference)
"""ConvLSTMCell Trainium2 kernel (8 NeuronCores, SPMD).

Problem (see reference): xi [4, 256, 16, 64, 64], W [256, 64, 3, 3], b [256]
  t=0:  gates from x0 directly, c0 = sig(i)*tanh(g), h0 = sig(o)*lrelu(c0)
  t>=1: tmp = conv3x3(h, W) + b + x_t;  c = sig(f)*c + sig(i)*tanh(g);
        h = sig(o)*lrelu(c)
Output: h stacked over t -> [4, 64, 16, 64, 64].

Sharding: 8 cores = (batch b, H-half). Each core computes a shrinking
redundant halo (region = its 32 rows + (15-t) extra rows toward the cut) so
there is NO inter-core communication. Bottom-half cores get their rows (and
W's ky axis) flipped host-side so all 8 cores run an identical program.

Per-core layout (channel-major):
  - out-channel permutation [i(0:64) f(64:128)] / [g(0:64) o(64:128)] so all
    elementwise ops are partition-band aligned (no cross-partition DVE ops).
  - conv = 7 matmul passes per 128-out-ch half per row-chunk:
      1 x K=128 identity (adds x_t into PSUM)
      3 x K=128 packed pair (partitions 64:128 = h padded, 0:64 = h shifted
        one col so one matmul covers taps (ky,kx=0)+(ky,kx=1))
      3 x K=64 singles (tap (ky,kx=2) read from the shifted copy)
  - gates: ACT sigmoid/tanh/lrelu with fused per-partition bias; c' = i*g+f*c
    via one DVE mul producing [ig; fc] + a stacked-identity matmul summing the
    bands on PE; h = o*l on GPSIMD. c state + h production all band-aligned.
  - fp32r matmuls (measured relerr ~1.5e-4 on HW).
"""
import numpy as np
from contextlib import ExitStack

import concourse.bacc as bacc
import concourse.tile as tile
from concourse import mybir
from concourse.bass_utils import run_bass_kernel_spmd

F32 = mybir.dt.float32
F32R = mybir.dt.float32r

B, CH4, T, HH, WW = 4, 256, 16, 64, 64
HID = 64
RG = 47            # region rows at t=0 (32 owned + 15 halo)
HP_R, WP = 48, 66  # padded h buffer rows/cols
CH_ROWS = 8        # rows per chunk
GROUP = 3          # chunks per psum group
NFLAT = RG * WW    # 3008

# weight blob columns
PK = [0, 256, 512]          # packed slabs ky=0,1,2  [128, 256]
SG = [768, 1024, 1280]      # single slabs ky=0,1,2  [64, 256] (upper rows 0)
IDC = 1536                  # identity 128           [128, 128]
IST = 1664                  # stacked identity       [128, 128]
WCOLS = 1792


def _emit_timestep_loop(nc, tc, pools, aps, repeats):
    consts, state, xp, hf, gp, op, lp, tp, psc = pools
    x_d, w_d, b_d, out_d = aps

    wb = consts.tile([128, WCOLS], F32R)
    bias = consts.tile([128, 2], F32)
    nc.sync.dma_start(out=wb, in_=w_d)
    nc.sync.dma_start(out=bias, in_=b_d)

    gc = state.tile([128, NFLAT], F32)       # [g ; c]
    hpadA = state.tile([128, HP_R * WP], F32R)
    hpadB = state.tile([128, HP_R * WP], F32R)
    hpads = [hpadA, hpadB]

    SIG = mybir.ActivationFunctionType.Sigmoid
    TANH = mybir.ActivationFunctionType.Tanh
    LRELU = mybir.ActivationFunctionType.Lrelu

    for _ in range(repeats):
        nc.vector.memset(hpadA.bitcast(F32), 0.0)
        nc.vector.memset(hpadB.bitcast(F32), 0.0)
        nc.vector.memset(gc[64:128, :], 0.0)

        for t in range(T):
            Ht = RG - t
            n_flat = Ht * WW
            nch = (Ht + CH_ROWS - 1) // CH_ROWS
            hp_w = hpads[t % 2]
            hp_r = hpads[(t + 1) % 2]
            hp3_w = hp_w.rearrange("p (r w) -> p r w", w=WP)
            hp3_r = hp_r.rearrange("p (r w) -> p r w", w=WP)

            xh = []
            for h in range(2):
                xt = xp.tile([128, NFLAT], F32R)
                x3 = xt.rearrange("p (r w) -> p r w", w=WW)
                nc.sync.dma_start(
                    out=x3[:, 0:Ht, :],
                    in_=x_d[t, 128 * h:128 * h + 128, 0:Ht, :])
                xh.append(xt)

            hfull = hf.tile([128, NFLAT], F32R)
            hf3 = hfull.rearrange("p (r w) -> p r w", w=WW)

            for g0 in range(0, nch, GROUP):
                chunks = list(range(g0, min(g0 + GROUP, nch)))
                ps = None
                if t > 0:
                    ps = [[psc.tile([128, CH_ROWS * WW], F32, tag="ps", name="ps")
                           for _ in chunks] for _ in range(2)]
                    for h in range(2):
                        hw = 128 * h
                        for ky in range(3):
                            for ci, c in enumerate(chunks):
                                rows = min(CH_ROWS, Ht - CH_ROWS * c)
                                n = rows * WW
                                r0 = CH_ROWS * c + ky
                                nc.tensor.matmul(
                                    ps[h][ci][:, 0:n],
                                    wb[:, PK[ky] + hw:PK[ky] + hw + 128],
                                    hp3_r[:, r0:r0 + rows, 0:64],
                                    start=(ky == 0), stop=False)
                        for ky in range(3):
                            for ci, c in enumerate(chunks):
                                rows = min(CH_ROWS, Ht - CH_ROWS * c)
                                n = rows * WW
                                r0 = CH_ROWS * c + ky
                                nc.tensor.matmul(
                                    ps[h][ci][:, 0:n],
                                    wb[0:64, SG[ky] + hw:SG[ky] + hw + 128],
                                    hp3_r[0:64, r0:r0 + rows, 1:65],
                                    start=False, stop=False)
                        for ci, c in enumerate(chunks):
                            rows = min(CH_ROWS, Ht - CH_ROWS * c)
                            n = rows * WW
                            nc.tensor.matmul(
                                ps[h][ci][:, 0:n],
                                wb[:, IDC:IDC + 128],
                                xh[h][:, 512 * c:512 * c + n],
                                start=False, stop=True)

                for ci, c in enumerate(chunks):
                    rows = min(CH_ROWS, Ht - CH_ROWS * c)
                    n = rows * WW
                    win = slice(512 * c, 512 * c + n)
                    if t > 0:
                        s0 = ps[0][ci][:, 0:n]
                        s1 = ps[1][ci][:, 0:n]
                        b0, b1 = bias[:, 0:1], bias[:, 1:2]
                    else:
                        s0 = xh[0][:, win]
                        s1 = xh[1][:, win]
                        b0, b1 = 0.0, 0.0

                    ifs = gp.tile([128, 512], F32)
                    nc.scalar.activation(ifs[:, 0:n], s0, SIG, bias=b0)
                    nc.scalar.activation(
                        gc[0:64, win], s1[0:64, :], TANH,
                        bias=(b1[0:64, :] if t > 0 else 0.0))
                    osb = op.tile([128, 512], F32)
                    nc.scalar.activation(
                        osb[64:128, 0:n], s1[64:128, :], SIG,
                        bias=(b1[64:128, :] if t > 0 else 0.0))

                    tmp = tp.tile([128, 512], F32R)
                    nc.vector.tensor_mul(tmp[:, 0:n], ifs[:, 0:n], gc[:, win])

                    if t > 0:
                        cps = ps[1][ci]
                    else:
                        cps = psc.tile([128, CH_ROWS * WW], F32, tag="ps", name="ps")
                    nc.tensor.matmul(cps[:, 0:n], wb[:, IST:IST + 128],
                                     tmp[:, 0:n], start=True, stop=True)

                    lsb = lp.tile([128, 512], F32)
                    nc.scalar.activation(lsb[64:128, 0:n], cps[64:128, 0:n],
                                         LRELU, alpha=0.01)
                    nc.vector.tensor_copy(gc[64:128, win], cps[64:128, 0:n])
                    nc.gpsimd.tensor_mul(hfull[64:128, win],
                                         osb[64:128, 0:n], lsb[64:128, 0:n])

                    r0 = CH_ROWS * c
                    if t < T - 1:
                        nc.sync.dma_start(
                            out=hp3_w[64:128, 1 + r0:1 + r0 + rows, 1:65],
                            in_=hf3[64:128, r0:r0 + rows, :])
                        nc.sync.dma_start(
                            out=hp3_w[0:64, 1 + r0:1 + r0 + rows, 0:64],
                            in_=hf3[64:128, r0:r0 + rows, :])
                    if r0 < 32:
                        srows = min(rows, 32 - r0)
                        nc.sync.dma_start(
                            out=out_d[:, t, r0:r0 + srows, :],
                            in_=hf3[64:128, r0:r0 + srows, :])


def build_nc(repeats=1):
    nc = bacc.Bacc("TRN2", target_bir_lowering=False, debug=False)
    x_d = nc.dram_tensor("x", [T, CH4, RG, WW], F32R,
                         kind="ExternalInput").ap()
    w_d = nc.dram_tensor("w", [128, WCOLS], F32R, kind="ExternalInput").ap()
    b_d = nc.dram_tensor("bias", [128, 2], F32, kind="ExternalInput").ap()
    out_d = nc.dram_tensor("out", [HID, T, 32, WW], F32R,
                           kind="ExternalOutput").ap()

    with tile.TileContext(nc) as tc, ExitStack() as ctx:
        consts = ctx.enter_context(tc.tile_pool(name="consts", bufs=1))
        state = ctx.enter_context(tc.tile_pool(name="state", bufs=1))
        xp = ctx.enter_context(tc.tile_pool(name="xp", bufs=6))
        hf = ctx.enter_context(tc.tile_pool(name="hf", bufs=2))
        gp = ctx.enter_context(tc.tile_pool(name="gp", bufs=4))
        op = ctx.enter_context(tc.tile_pool(name="op", bufs=3))
        lp = ctx.enter_context(tc.tile_pool(name="lp", bufs=3))
        tp = ctx.enter_context(tc.tile_pool(name="tp", bufs=4))
        psc = ctx.enter_context(
            tc.tile_pool(name="psc", bufs=8, space="PSUM"))
        _emit_timestep_loop(
            nc, tc, (consts, state, xp, hf, gp, op, lp, tp, psc),
            (x_d, w_d, b_d, out_d), repeats)
    nc.compile()
    return nc


def _prep_core_inputs(xi, W, b):
    """Host-side shard prep. Returns list of 8 in_maps."""
    # out-channel permutation: [i f g o]
    perm = np.concatenate([np.arange(0, 128), np.arange(192, 256),
                           np.arange(128, 192)])
    Wp = W[perm]                      # [256, 64, 3, 3]
    bp = b[perm]
    bias_blob = np.stack([bp[0:128], bp[128:256]], axis=1).astype(np.float32)
    bias_blob = np.ascontiguousarray(bias_blob)  # [128, 2]

    def wblob(Wv):
        wb = np.zeros((128, WCOLS), np.float32)
        for ky in range(3):
            wb[0:64, PK[ky]:PK[ky] + 256] = Wv[:, :, ky, 1].T
            wb[64:128, PK[ky]:PK[ky] + 256] = Wv[:, :, ky, 0].T
            wb[0:64, SG[ky]:SG[ky] + 256] = Wv[:, :, ky, 2].T
        wb[:, IDC:IDC + 128] = np.eye(128)
        ist = np.zeros((128, 128), np.float32)
        ist[0:64, 64:128] = np.eye(64)
        ist[64:128, 64:128] = np.eye(64)
        wb[:, IST:IST + 128] = ist
        return wb

    wb_top = wblob(Wp)
    wb_bot = wblob(Wp[:, :, ::-1, :])  # ky flipped for row-flipped cores

    in_maps = []
    for core in range(8):
        bb, half = divmod(core, 2)
        xs = xi[bb][perm]                      # [256, 16, 64, 64]
        if half == 0:
            xs = xs[:, :, 0:RG, :]
        else:
            xs = xs[:, :, ::-1, :][:, :, 0:RG, :]
        xs = np.ascontiguousarray(xs.transpose(1, 0, 2, 3))  # [16,256,47,64]
        in_maps.append({
            "x": xs.astype(np.float32),
            "w": (wb_top if half == 0 else wb_bot),
            "bias": bias_blob,
        })
    return in_maps


_NC_CACHE = {}


def kernel(xi, W, b):
    xi = np.asarray(xi, dtype=np.float32)
    W = np.asarray(W, dtype=np.float32)
    b = np.asarray(b, dtype=np.float32)
    if "nc" not in _NC_CACHE:
        _NC_CACHE["nc"] = build_nc(repeats=1)
    nc = _NC_CACHE["nc"]
    in_maps = _prep_core_inputs(xi, W, b)
    res = run_bass_kernel_spmd(nc, in_maps, list(range(8)), trace=False)
    out = np.empty((B, HID, T, HH, WW), np.float32)
    for core in range(8):
        bb, half = divmod(core, 2)
        o = res.results[core]["out"]          # [64, 16, 32, 64]
        o = np.moveaxis(o, 1, 1)              # [hid, T, 32, W]
        if half == 0:
            out[bb, :, :, 0:32, :] = o
        else:
            out[bb, :, :, 32:64, :] = o[:, :, ::-1, :]
    return out



# revision 5
# speedup vs baseline: 4.8937x; 4.8937x over previous
"""ConvLSTMCell Trainium2 kernel v2 (8 NeuronCores, SPMD).

Same sharding/layout as v1 (see kernel.py docstring), with the DMA-issue
bottleneck removed:
  - Prelu (parametric relu) instead of Lrelu: lives in the same activation
    table set as Sigmoid/Tanh -> no per-chunk ACT table reloads.
  - h is written straight into the padded h buffer by compute engines
    (Pool -> band 64:128 at cols 1:65; DVE cross-partition-write -> band
    0:64 at cols 0:64) instead of 2 SBUF->SBUF DMAs per chunk.
  - output to HBM is one DMA per timestep reading the padded buffer
    (rows 1:33) instead of one DMA per chunk.
DMA count per pass: 32 x-loads + 16 out + 2 consts (vs 262 in v1); each
dma_start holds the single shared HWDGE unit ~625 ns, so issue pressure
was the critical path.
"""
import os
import numpy as np
import ml_dtypes
from contextlib import ExitStack

CFG_GROUP = int(os.environ.get("K5_GROUP", "2"))
CFG_ORDER = os.environ.get("K5_ORDER", "cmajor")   # hmajor | cmajor
CFG_LRELU = os.environ.get("K5_LRELU", "act")     # act | dve (pool lacks TensorScalarPtr)
CFG_CCOPY = os.environ.get("K5_CCOPY", "dve")      # dve | act
CFG_XADD = os.environ.get("K5_XADD", "stt")        # id | stt
CFG_HMUL = os.environ.get("K5_HMUL", "dve")        # pool | dve
CFG_BAND0 = os.environ.get("K5_BAND0", "dve")      # dve | act | pool
CFG_TMP = os.environ.get("K5_TMP", "pool")         # dve | pool (pool: SBUF-only ops)

import concourse.bacc as bacc
import concourse.tile as tile
from concourse import mybir
from concourse.bass_utils import run_bass_kernel_spmd

F32 = mybir.dt.float32
F32R = mybir.dt.float32r
BF16 = mybir.dt.bfloat16

B, CH4, T, HH, WW = 4, 256, 16, 64, 64
HID = 64
RG = 47            # region rows at t=0 (32 owned + 15 halo)
HP_R, WP = 48, 66  # padded h buffer rows/cols
CH_ROWS = 8        # rows per chunk
GROUP = None       # set from CFG_GROUP in build_nc
NFLAT = RG * WW    # 3008

# weight blob columns
PK = [0, 256, 512]          # packed slabs ky=0,1,2  [128, 256]
SG = [768, 1024, 1280]      # single slabs ky=0,1,2  [64, 256] (upper rows 0)
IDC = 1536                  # identity 128           [128, 128]
IST = 1664                  # stacked identity       [128, 128]
WCOLS = 1792


def _emit_timestep_loop(nc, tc, pools, aps, repeats):
    consts, state, xp, gp, op, lp, tp, smp, psc = pools
    x_d, id_d, w_d, b_d, out_d = aps

    wb = consts.tile([128, WCOLS], F32R)
    idb = consts.tile([128, 128], BF16)
    bias = consts.tile([128, 2], F32)
    nc.sync.dma_start(out=wb, in_=w_d)
    nc.sync.dma_start(out=idb, in_=id_d)
    nc.sync.dma_start(out=bias, in_=b_d)

    gc = state.tile([128, NFLAT], F32)       # [g ; c]
    hpadA = state.tile([128, HP_R * WP], F32R)
    hpadB = state.tile([128, HP_R * WP], F32R)
    hpads = [hpadA, hpadB]

    SIG = mybir.ActivationFunctionType.Sigmoid
    TANH = mybir.ActivationFunctionType.Tanh
    LRELU = mybir.ActivationFunctionType.Prelu  # same curve as leaky relu,
    # but lives in the sigmoid_and_others table set -> no table reloads

    for _ in range(repeats):
        nc.vector.memset(hpadA.bitcast(F32), 0.0)
        nc.vector.memset(hpadB.bitcast(F32), 0.0)
        nc.vector.memset(gc[64:128, :], 0.0)

        for t in range(T):
            Ht = RG - t
            n_flat = Ht * WW
            nch = (Ht + CH_ROWS - 1) // CH_ROWS
            hp_w = hpads[t % 2]
            hp_r = hpads[(t + 1) % 2]
            hp3_w = hp_w.rearrange("p (r w) -> p r w", w=WP)
            hp3_r = hp_r.rearrange("p (r w) -> p r w", w=WP)

            xh = []
            for h in range(2):
                xt = xp.tile([128, NFLAT], BF16)
                x3 = xt.rearrange("p (r w) -> p r w", w=WW)
                nc.sync.dma_start(
                    out=x3[:, 0:Ht, :],
                    in_=x_d[t, 128 * h:128 * h + 128, 0:Ht, :])
                xh.append(xt)

            for g0 in range(0, nch, CFG_GROUP):
                chunks = list(range(g0, min(g0 + CFG_GROUP, nch)))
                ps = None
                if t > 0:
                    ps = [[psc.tile([128, CH_ROWS * WW], F32, tag="ps", name="ps")
                           for _ in chunks] for _ in range(2)]

                    def emit_mm(h, ci, c, kind, ky=0):
                        hw = 128 * h
                        rows = min(CH_ROWS, Ht - CH_ROWS * c)
                        n = rows * WW
                        r0 = CH_ROWS * c + ky
                        if kind == "pk":
                            nc.tensor.matmul(
                                ps[h][ci][:, 0:n],
                                wb[:, PK[ky] + hw:PK[ky] + hw + 128],
                                hp3_r[:, r0:r0 + rows, 0:64],
                                start=(ky == 0), stop=False)
                        elif kind == "sg":
                            nc.tensor.matmul(
                                ps[h][ci][:, 0:n],
                                wb[0:64, SG[ky] + hw:SG[ky] + hw + 128],
                                hp3_r[0:64, r0:r0 + rows, 1:65],
                                start=False,
                                stop=(ky == 2 and (CFG_XADD == "stt" or
                                      (CFG_XADD == "mix" and h == 0))))
                        else:
                            nc.tensor.matmul(
                                ps[h][ci][:, 0:n],
                                idb,
                                xh[h][:, 512 * c:512 * c + n],
                                start=False, stop=True)

                    if CFG_ORDER == "hmajor":
                        for h in range(2):
                            for ky in range(3):
                                for ci, c in enumerate(chunks):
                                    emit_mm(h, ci, c, "pk", ky)
                            for ky in range(3):
                                for ci, c in enumerate(chunks):
                                    emit_mm(h, ci, c, "sg", ky)
                            if CFG_XADD == "id" or (CFG_XADD == "mix"
                                                     and h == 1):
                                for ci, c in enumerate(chunks):
                                    emit_mm(h, ci, c, "id")
                    else:  # cmajor: finish both banks of a chunk ASAP
                        for ci, c in enumerate(chunks):
                            for h in range(2):
                                for ky in range(3):
                                    emit_mm(h, ci, c, "pk", ky)
                                for ky in range(3):
                                    emit_mm(h, ci, c, "sg", ky)
                                if CFG_XADD == "id" or (CFG_XADD == "mix"
                                                        and h == 1):
                                    emit_mm(h, ci, c, "id")

                for ci, c in enumerate(chunks):
                    rows = min(CH_ROWS, Ht - CH_ROWS * c)
                    n = rows * WW
                    win = slice(512 * c, 512 * c + n)
                    if t > 0 and CFG_XADD in ("stt", "mix"):
                        sm0 = smp.tile([128, 512], F32, name="sm0")
                        nc.vector.scalar_tensor_tensor(
                            sm0[:, 0:n], ps[0][ci][:, 0:n], bias[:, 0:1],
                            xh[0][:, win],
                            mybir.AluOpType.add, mybir.AluOpType.add)
                        s0 = sm0[:, 0:n]
                        b0 = 0.0
                        if CFG_XADD == "stt":
                            sm1 = smp.tile([128, 512], F32, name="sm1")
                            nc.vector.scalar_tensor_tensor(
                                sm1[:, 0:n], ps[1][ci][:, 0:n], bias[:, 1:2],
                                xh[1][:, win],
                                mybir.AluOpType.add, mybir.AluOpType.add)
                            s1 = sm1[:, 0:n]
                            b1 = 0.0
                        else:
                            s1 = ps[1][ci][:, 0:n]
                            b1 = bias[:, 1:2]
                    elif t > 0:
                        s0 = ps[0][ci][:, 0:n]
                        s1 = ps[1][ci][:, 0:n]
                        b0, b1 = bias[:, 0:1], bias[:, 1:2]
                    else:
                        s0 = xh[0][:, win]
                        s1 = xh[1][:, win]
                        b0, b1 = 0.0, 0.0

                    b1_lo = 0.0 if isinstance(b1, float) else b1[0:64, :]
                    b1_hi = 0.0 if isinstance(b1, float) else b1[64:128, :]
                    ifs = gp.tile([128, 512], F32)
                    nc.scalar.activation(ifs[:, 0:n], s0, SIG, bias=b0)
                    nc.scalar.activation(
                        gc[0:64, win], s1[0:64, :], TANH, bias=b1_lo)
                    osb = op.tile([128, 512], F32)
                    nc.scalar.activation(
                        osb[64:128, 0:n], s1[64:128, :], SIG, bias=b1_hi)

                    tmp = tp.tile([128, 512], F32R)
                    if CFG_TMP == "pool" and (t > 0 and CFG_XADD == "stt"):
                        nc.gpsimd.tensor_mul(tmp[:, 0:n], ifs[:, 0:n],
                                             gc[:, win])
                    else:
                        nc.vector.tensor_mul(tmp[:, 0:n], ifs[:, 0:n],
                                             gc[:, win])

                    if t > 0:
                        cps = ps[1][ci]
                    else:
                        cps = psc.tile([128, CH_ROWS * WW], F32, tag="ps", name="ps")
                    nc.tensor.matmul(cps[:, 0:n], wb[:, IST:IST + 128],
                                     tmp[:, 0:n], start=True, stop=True)

                    # c state to SBUF first; lrelu may then read SBUF
                    if CFG_CCOPY == "dve":
                        nc.vector.tensor_copy(gc[64:128, win], cps[64:128, 0:n])
                    elif CFG_CCOPY == "act":
                        nc.scalar.activation(gc[64:128, win], cps[64:128, 0:n],
                                             mybir.ActivationFunctionType.Copy)
                    else:  # pool cannot read PSUM; fall back to DVE
                        nc.vector.tensor_copy(gc[64:128, win], cps[64:128, 0:n])
                    lsb = lp.tile([128, 512], F32)
                    if CFG_LRELU == "pool":
                        # Pool cannot touch PSUM: read c' from gc (SBUF)
                        nc.gpsimd.scalar_tensor_tensor(
                            lsb[64:128, 0:n], gc[64:128, win], 0.01,
                            gc[64:128, win],
                            mybir.AluOpType.mult, mybir.AluOpType.max)
                    elif CFG_LRELU == "dve":
                        nc.vector.scalar_tensor_tensor(
                            lsb[64:128, 0:n], gc[64:128, win], 0.01,
                            gc[64:128, win],
                            mybir.AluOpType.mult, mybir.AluOpType.max)
                    else:
                        nc.scalar.activation(lsb[64:128, 0:n], cps[64:128, 0:n],
                                             LRELU, alpha=0.01)

                    # h = o * lrelu(c'), written straight into the padded h
                    # buffer: Pool -> band 64:128 (cols 1:65), DVE cross-
                    # partition-write -> band 0:64 (cols 0:64, the per-col
                    # shifted copy the packed matmuls read).
                    r0 = CH_ROWS * c
                    if CFG_HMUL == "pool":
                        nc.gpsimd.tensor_mul(
                            hp3_w[64:128, 1 + r0:1 + r0 + rows, 1:65],
                            osb[64:128, 0:n], lsb[64:128, 0:n])
                    else:
                        nc.vector.tensor_mul(
                            hp3_w[64:128, 1 + r0:1 + r0 + rows, 1:65],
                            osb[64:128, 0:n], lsb[64:128, 0:n])
                    if CFG_BAND0 == "dve":
                        nc.vector.tensor_copy(
                            hp3_w[0:64, 1 + r0:1 + r0 + rows, 0:64],
                            hp3_w[64:128, 1 + r0:1 + r0 + rows, 1:65])
                    elif CFG_BAND0 == "act":
                        nc.scalar.activation(
                            hp3_w[0:64, 1 + r0:1 + r0 + rows, 0:64].bitcast(F32),
                            hp3_w[64:128, 1 + r0:1 + r0 + rows, 1:65].bitcast(F32),
                            mybir.ActivationFunctionType.Copy)
                    else:
                        nc.gpsimd.tensor_copy(
                            hp3_w[0:64, 1 + r0:1 + r0 + rows, 0:64],
                            hp3_w[64:128, 1 + r0:1 + r0 + rows, 1:65])

            # owned rows 0:32 -> HBM, straight from the padded buffer
            nc.sync.dma_start(
                out=out_d[:, t, :, :],
                in_=hp3_w[64:128, 1:33, 1:65])


def build_nc(repeats=1):
    nc = bacc.Bacc("TRN2", target_bir_lowering=False, debug=False)
    x_d = nc.dram_tensor("x", [T, CH4, RG, WW], BF16,
                         kind="ExternalInput").ap()
    w_d = nc.dram_tensor("w", [128, WCOLS], F32R, kind="ExternalInput").ap()
    id_d = nc.dram_tensor("idb", [128, 128], BF16, kind="ExternalInput").ap()
    b_d = nc.dram_tensor("bias", [128, 2], F32, kind="ExternalInput").ap()
    out_d = nc.dram_tensor("out", [HID, T, 32, WW], F32R,
                           kind="ExternalOutput").ap()

    with tile.TileContext(nc) as tc, ExitStack() as ctx:
        consts = ctx.enter_context(tc.tile_pool(name="consts", bufs=1))
        state = ctx.enter_context(tc.tile_pool(name="state", bufs=1))
        xp = ctx.enter_context(tc.tile_pool(name="xp", bufs=6))
        gp = ctx.enter_context(tc.tile_pool(name="gp", bufs=4))
        op = ctx.enter_context(tc.tile_pool(name="op", bufs=3))
        lp = ctx.enter_context(tc.tile_pool(name="lp", bufs=3))
        tp = ctx.enter_context(tc.tile_pool(name="tp", bufs=4))
        smp = ctx.enter_context(tc.tile_pool(name="smp", bufs=4))
        psc = ctx.enter_context(
            tc.tile_pool(name="psc", bufs=8, space="PSUM"))
        _emit_timestep_loop(
            nc, tc, (consts, state, xp, gp, op, lp, tp, smp, psc),
            (x_d, id_d, w_d, b_d, out_d), repeats)
    nc.compile()
    return nc


def _prep_core_inputs(xi, W, b):
    """Host-side shard prep. Returns list of 8 in_maps."""
    # out-channel permutation: [i f g o]
    perm = np.concatenate([np.arange(0, 128), np.arange(192, 256),
                           np.arange(128, 192)])
    Wp = W[perm]                      # [256, 64, 3, 3]
    bp = b[perm]
    bias_blob = np.stack([bp[0:128], bp[128:256]], axis=1).astype(np.float32)
    bias_blob = np.ascontiguousarray(bias_blob)  # [128, 2]

    def wblob(Wv):
        wb = np.zeros((128, WCOLS), np.float32)
        for ky in range(3):
            wb[0:64, PK[ky]:PK[ky] + 256] = Wv[:, :, ky, 1].T
            wb[64:128, PK[ky]:PK[ky] + 256] = Wv[:, :, ky, 0].T
            wb[0:64, SG[ky]:SG[ky] + 256] = Wv[:, :, ky, 2].T
        wb[:, IDC:IDC + 128] = np.eye(128)
        ist = np.zeros((128, 128), np.float32)
        ist[0:64, 64:128] = np.eye(64)
        ist[64:128, 64:128] = np.eye(64)
        wb[:, IST:IST + 128] = ist
        return wb

    wb_top = wblob(Wp)
    wb_bot = wblob(Wp[:, :, ::-1, :])  # ky flipped for row-flipped cores

    in_maps = []
    for core in range(8):
        bb, half = divmod(core, 2)
        xs = xi[bb][perm]                      # [256, 16, 64, 64]
        if half == 0:
            xs = xs[:, :, 0:RG, :]
        else:
            xs = xs[:, :, ::-1, :][:, :, 0:RG, :]
        xs = np.ascontiguousarray(xs.transpose(1, 0, 2, 3))  # [16,256,47,64]
        in_maps.append({
            "x": xs.astype(ml_dtypes.bfloat16),
            "idb": np.eye(128, dtype=ml_dtypes.bfloat16),
            "w": (wb_top if half == 0 else wb_bot),
            "bias": bias_blob,
        })
    return in_maps


_NC_CACHE = {}


def kernel(xi, W, b):
    xi = np.asarray(xi, dtype=np.float32)
    W = np.asarray(W, dtype=np.float32)
    b = np.asarray(b, dtype=np.float32)
    if "nc" not in _NC_CACHE:
        _NC_CACHE["nc"] = build_nc(repeats=1)
    nc = _NC_CACHE["nc"]
    in_maps = _prep_core_inputs(xi, W, b)
    res = run_bass_kernel_spmd(nc, in_maps, list(range(8)), trace=False)
    out = np.empty((B, HID, T, HH, WW), np.float32)
    for core in range(8):
        bb, half = divmod(core, 2)
        o = res.results[core]["out"]          # [64, 16, 32, 64]
        if half == 0:
            out[bb, :, :, 0:32, :] = o
        else:
            out[bb, :, :, 32:64, :] = o[:, :, ::-1, :]
    return out
